# revision 19
# baseline (speedup 1.0000x reference)
"""Styled 3D conv (StyleGAN-style modulated conv3d) on 8 Trainium2 NeuronCores.

Reference computation:
  st = s @ style_weight.T + style_bias                 (N, Cin)
  w  = weight[None] * st[:, None, :, None*3]           (N, Cout, Cin, 3,3,3)
  w  = w * rsqrt(sum(w^2 over (Cin,kd,kh,kw)) + eps)   per-sample demodulated
  y  = grouped_conv3d(x, w, groups=N, VALID) + bias    (N, Cout, 62,62,62)

Shapes: x (4, 64, 64, 64, 64) f32, y (4, 64, 62, 62, 62) f32.

Strategy (8 cores = 4 samples x 2 depth-halves), Winograd F(2,3) along W:
  - Host: modulated weights -> G-transform along kw: u_t (4 transform
    points), packed into 24 lhsT blocks (kh x t x slot-var); input x is
    cast bf16 and W-transformed on host into 4 half-width copies
    x~_t[ci, plane, t, h, wt] (all +-1 combos, wt in [0,31)).
  - Each core gets 34 input planes' x~ and produces 31.5 output planes
    (odd cores depth+height mirrored, merged on gather — W untouched,
    so the W-transform commutes with the mirror).
  - SBUF x~ slot q = [128 part, 4, 64, 31] bf16: partitions 0:64 =
    (ci, plane 2q), 64:128 = (ci, plane 2q+1); ring of 5 slots.
  - Output plane pair (2j, 2j+1), M=128 packing (psum 0:64 -> y[d] Cout,
    64:128 -> y[d+1]) by 24 streams: 2 slots x 3 kh x 4 t, K=128 =
    (2 planes x ci), same 6/8-quadrant kd band as direct (75% PE).
    Winograd replaces 3 kw-taps at N=62 with 4 t-points at N=31
    => 2/3 the PE cycles of the direct kernel.
  - Per pair, 62 output rows = 4 chunks of {16,16,16,14}; per chunk 4
    PSUM banks hold m_0..m_3 [128, rows, 31]; 2 chunks in flight.
  - Drain (inverse transform, fp32): y_even = m0+m1+m2+bias,
    y_odd = m1-m2-m3+bias via DVE, written interleaved (stride 2) into
    SBUF staging, then DMA to DRAM.

Measured baseline (direct bf16): 483 us. This kernel targets ~330 us.
"""

import numpy as np
import ml_dtypes

import concourse.mybir as mybir
import concourse.tile as tile
from concourse import bacc
from concourse.bass_utils import run_bass_kernel_spmd

EPS = 1e-8
N_CORES = 8
N, CIN, COUT, D = 4, 64, 64, 64
DO = D - 2              # 62 output planes/rows/cols
WT = 31                 # Winograd w-tiles per row
T = 4                   # transform points
PLANES_IN = 34          # input planes per core
PAIRS = 16              # output plane pairs per core (32 planes)
SLOTS = PLANES_IN // 2  # 17
XRING = 6               # x~ slot ring buffers
CHUNK_ROWS = (16, 16, 16, 14)
CHUNK_OFF = (0, 16, 32, 48)
BF16 = mybir.dt.bfloat16
F32 = mybir.dt.float32
ADD = mybir.AluOpType.add
SUB = mybir.AluOpType.subtract

_compiled = {}

# stream order per pair: var-major, then kh, then t
_STREAMS = [(var, kh, t) for var in range(2) for kh in range(3)
            for t in range(T)]


def _build_nc():
    nc = bacc.Bacc("TRN2", target_bir_lowering=False, debug=False,
                   num_devices=N_CORES)
    xt_d = nc.dram_tensor("xt", [CIN, PLANES_IN, T, D, WT], BF16,
                          kind="ExternalInput").ap()
    wts = nc.dram_tensor("wts", [128, 24 * 128], BF16,
                         kind="ExternalInput").ap()
    b128 = nc.dram_tensor("b128", [128, 1], F32, kind="ExternalInput").ap()
    y = nc.dram_tensor("y", [COUT, 2 * PAIRS, DO * DO], F32,
                       kind="ExternalOutput").ap()

    with tile.TileContext(nc) as tc:
        with (
            tc.tile_pool(name="wp", bufs=1) as wpool,
            tc.tile_pool(name="xp", bufs=XRING) as xpool,
            tc.tile_pool(name="ps", bufs=8, space="PSUM") as pspool,
            tc.tile_pool(name="st", bufs=4) as stpool,
            tc.tile_pool(name="tp", bufs=4) as tmppool,
        ):
            w_sb = wpool.tile([128, 24 * 128], BF16)
            # weights go on gpsimd's queue (idle at the head) so they don't
            # delay the input slots that gate the first matmuls
            for i in range(6):
                o = i * 512
                nc.gpsimd.dma_start(w_sb[:, o:o + 512], wts[:, o:o + 512])
            bias_sb = wpool.tile([128, 1], F32)
            nc.gpsimd.dma_start(bias_sb[:, :], b128[:, :])

            slots = {}

            def load_slot(q, row_split=False):
                t = xpool.tile([128, T, D, WT], BF16, tag="xt",
                               name=f"xt_{q}")
                # spread halves over the three DMA-capable engines' queue
                # sets — one set alone (~190 GB/s) starves the head
                for half, pl in ((0, 2 * q), (1, 2 * q + 1)):
                    p0, p1 = 64 * half, 64 * half + 64
                    eng = (nc.sync, nc.scalar, nc.gpsimd)[(2 * q + half) % 3]
                    ngrp = 4 if row_split else 2
                    for g in range(ngrp):
                        r = (D // ngrp) * g
                        r1 = r + D // ngrp
                        eng.dma_start(t[p0:p1, :, r:r1, :],
                                      xt_d[:, pl, :, r:r1, :])
                slots[q] = t

            for q in range(XRING):
                load_slot(q, row_split=(q < 2))

            # PE warmup: hold the PE busy with garbage matmuls while the
            # first slots' DMAs are in flight so the HAM clock gate opens
            # (full 2.4 GHz needs ~3.4us of sustained activity).
            warm_src = wpool.tile([128, 496], BF16, name="warm_src")
            nc.vector.memset(warm_src[:, :], 0.0)
            warm_ps = pspool.tile([128, 16, WT], F32, tag="ps",
                                  name="warm_ps")
            for _ in range(250):
                nc.tensor.matmul(warm_ps[:, 0:2, :], warm_src[:, 0:128],
                                 warm_src[:, 128:190], start=True, stop=True)

            IDENT = mybir.ActivationFunctionType.Identity

            def drain(j, c, psums):
                rows = CHUNK_ROWS[c]
                stage = stpool.tile([128, 16, DO], F32, tag="stage")
                t1 = tmppool.tile([128, 16, WT], F32, tag="tmp")
                t2 = tmppool.tile([128, 16, WT], F32, tag="tmp")
                g1 = tmppool.tile([128, 16, WT], F32, tag="tmp")
                g2 = tmppool.tile([128, 16, WT], F32, tag="tmp")
                m0, m1, m2, m3 = psums
                # a = m1 + bias (ACT); b = -m3 (ACT); gpsimd can't read PSUM
                nc.scalar.activation(t1[:, :rows], m1[:, :rows], IDENT,
                                     bias=bias_sb)
                nc.scalar.activation(g1[:, :rows], m3[:, :rows],
                                     mybir.ActivationFunctionType.Copy,
                                     scale=-1.0)
                # y_even = (a + m0) + m2
                nc.vector.tensor_tensor(out=t2[:, :rows], in0=t1[:, :rows],
                                        in1=m0[:, :rows], op=ADD)
                nc.vector.tensor_tensor(out=stage[:, :rows, 0:DO:2],
                                        in0=t2[:, :rows], in1=m2[:, :rows],
                                        op=ADD)
                # y_odd = (a - m2) + b
                nc.vector.tensor_tensor(out=g2[:, :rows], in0=t1[:, :rows],
                                        in1=m2[:, :rows], op=SUB)
                nc.gpsimd.tensor_tensor(out=stage[:, :rows, 1:DO:2],
                                        in0=g2[:, :rows], in1=g1[:, :rows],
                                        op=ADD)
                # alternate output DMAs between gpsimd's and vector's DGE
                # queue sets — keeps sync free for inputs and halves the
                # end-of-kernel queue flush
                o = CHUNK_OFF[c] * DO
                n = rows * DO
                eng = nc.gpsimd if c % 2 == 0 else nc.scalar
                eng.dma_start(y[:, 2 * j, o:o + n], stage[0:64, :rows])
                eng.dma_start(y[:, 2 * j + 1, o:o + n],
                              stage[64:128, :rows])

            for j in range(PAIRS):
                last = j == PAIRS - 1
                # chunk-major: drains overlap the next chunk's matmuls
                for c in range(2 if last else 4):
                    psums = [pspool.tile([128, 16, WT], F32, tag="ps",
                                         name=f"ps_{j}_{c}_{t}")
                             for t in range(T)]
                    rows = CHUNK_ROWS[c]
                    for var, kh, t in _STREAMS:
                        blk = (kh * 4 + t) * 2 + var
                        lhsT = w_sb[:, blk * 128:(blk + 1) * 128]
                        sl = slots[j + var]
                        r0 = CHUNK_OFF[c] + kh
                        nc.tensor.matmul(
                            psums[t][:, :rows], lhsT,
                            sl[:, t, r0:r0 + rows, :],
                            start=(var == 0 and kh == 0),
                            stop=(var == 1 and kh == 2))
                    drain(j, c, psums)
                # prefetch the slot this pair's buffer slot frees up
                q = j + XRING
                if q < SLOTS:
                    load_slot(q)
    nc.compile()
    return nc


def _modulated_weights(s_n, style_weight, style_bias, weight):
    st = s_n.astype(np.float32) @ style_weight.T.astype(np.float32) + style_bias
    w = weight * st[None, :, None, None, None]
    demod = 1.0 / np.sqrt(np.sum(w * w, axis=(1, 2, 3, 4)) + EPS)
    return w * demod[:, None, None, None, None]

_G = np.array([[1, 0, 0], [.5, .5, .5], [.5, -.5, .5], [0, 0, 1]], np.float32)


def _build_lhsT(wmod):
    """(24, 128, 128) fp32 blocks: blk=(kh*4+t)*2+var; lhsT[k=(half,ci), m=(colhalf,co)]."""
    ut = np.einsum('tw,oidhw->toidh', _G, wmod)
    out = np.zeros((3, T, 2, 128, 128), np.float32)
    for kh in range(3):
        for t in range(T):
            wt = ut[t][:, :, :, kh]            # (co, ci, kd)
            A = out[kh, t, 0]
            B = out[kh, t, 1]
            A[0:64, 0:64] = wt[:, :, 0].T      # lower -> y[d],   kd0
            A[64:128, 0:64] = wt[:, :, 1].T    # upper -> y[d],   kd1
            A[64:128, 64:128] = wt[:, :, 0].T  # upper -> y[d+1], kd0
            B[0:64, 0:64] = wt[:, :, 2].T      # lower -> y[d],   kd2
            B[0:64, 64:128] = wt[:, :, 1].T    # lower -> y[d+1], kd1
            B[64:128, 64:128] = wt[:, :, 2].T  # upper -> y[d+1], kd2
    return out.reshape(24, 128, 128)


def _wino_xt(xs_bf):
    """(Ci, P, D, D) bf16 -> (Ci, P, 4, D, 31) bf16 W-transform."""
    xw = xs_bf.astype(np.float32)
    i = 2 * np.arange(WT)
    t0 = xw[..., i] - xw[..., i + 2]
    t1 = xw[..., i + 1] + xw[..., i + 2]
    t2 = xw[..., i + 2] - xw[..., i + 1]
    t3 = xw[..., i + 1] - xw[..., i + 3]
    return np.ascontiguousarray(
        np.stack([t0, t1, t2, t3], axis=2).astype(ml_dtypes.bfloat16))


def _prepare_in_maps(x, s, style_weight, style_bias, weight, bias):
    bias128 = np.concatenate([bias.reshape(COUT), bias.reshape(COUT)])
    bias128 = np.ascontiguousarray(bias128.reshape(128, 1), np.float32)

    x_bf = x.astype(ml_dtypes.bfloat16)
    in_maps = []
    for core in range(N_CORES):
        n, half = divmod(core, 2)
        wmod = _modulated_weights(s[n], style_weight, style_bias, weight)
        if half == 0:
            xs = x_bf[n][:, 0:PLANES_IN]
        else:
            # mirrored shard: flip depth + height; kernel taps flip too,
            # so the same program computes the flipped top half
            xs = x_bf[n][:, D - PLANES_IN:D][:, ::-1, ::-1, :]
            wmod = wmod[:, :, ::-1, ::-1, :]
        lhsT = _build_lhsT(np.ascontiguousarray(wmod))  # (24, 128, 128)
        wts = np.ascontiguousarray(
            lhsT.transpose(1, 0, 2).reshape(128, 24 * 128)
        ).astype(ml_dtypes.bfloat16)
        in_maps.append({"xt": _wino_xt(np.ascontiguousarray(xs)),
                        "wts": wts, "b128": bias128})
    return in_maps


def kernel(x, s, style_weight, style_bias, weight, bias):
    x = np.asarray(x)
    s = np.asarray(s)
    style_weight = np.asarray(style_weight, np.float32)
    style_bias = np.asarray(style_bias, np.float32)
    weight = np.asarray(weight, np.float32)
    bias = np.asarray(bias, np.float32)

    if "nc" not in _compiled:
        _compiled["nc"] = _build_nc()
    nc = _compiled["nc"]

    in_maps = _prepare_in_maps(x, s, style_weight, style_bias, weight, bias)
    res = run_bass_kernel_spmd(nc, in_maps, core_ids=list(range(N_CORES)))

    y = np.empty((N, COUT, DO, DO, DO), np.float32)
    for core in range(N_CORES):
        n, half = divmod(core, 2)
        ys = np.asarray(res.results[core]["y"]).astype(np.float32)
        ys = ys.reshape(COUT, 2 * PAIRS, DO, DO)
        if half == 0:
            # planes 0..29 full; planes 30,31 rows 0..31 only
            y[n][:, 0:30] = ys[:, 0:30]
            y[n][:, 30:32, 0:32] = ys[:, 30:32, 0:32]
        else:
            # un-mirror: ysf[p', r'] = global (plane 30+p', row r')
            ysf = ys[:, ::-1, ::-1, :]
            y[n][:, 32:DO] = ysf[:, 2:32]
            y[n][:, 30:32, 32:DO] = ysf[:, 0:2, 32:DO]
    return y


# revision 24
# speedup vs baseline: 1.0249x; 1.0249x over previous
"""Styled 3D conv (StyleGAN-style modulated conv3d) on 8 Trainium2 NeuronCores.

Reference computation:
  st = s @ style_weight.T + style_bias                 (N, Cin)
  w  = weight[None] * st[:, None, :, None*3]           (N, Cout, Cin, 3,3,3)
  w  = w * rsqrt(sum(w^2 over (Cin,kd,kh,kw)) + eps)   per-sample demodulated
  y  = grouped_conv3d(x, w, groups=N, VALID) + bias    (N, Cout, 62,62,62)

Shapes: x (4, 64, 64, 64, 64) f32, y (4, 64, 62, 62, 62) f32.

Strategy (8 cores = 4 samples x 2 depth-halves), Winograd F(2,3) along W:
  - Host: modulated weights -> G-transform along kw: u_t (4 transform
    points), packed into 24 lhsT blocks (kh x t x slot-var); input x is
    cast bf16 and W-transformed on host into 4 half-width copies
    x~_t[ci, plane, t, h, wt] (all +-1 combos, wt in [0,31)).
  - Each core gets 34 input planes' x~ and produces 31.5 output planes
    (odd cores depth+height mirrored, merged on gather — W untouched,
    so the W-transform commutes with the mirror).
  - SBUF x~ slot q = [128 part, 4, 64, 31] bf16: partitions 0:64 =
    (ci, plane 2q), 64:128 = (ci, plane 2q+1); ring of 5 slots.
  - Output plane pair (2j, 2j+1), M=128 packing (psum 0:64 -> y[d] Cout,
    64:128 -> y[d+1]) by 24 streams: 2 slots x 3 kh x 4 t, K=128 =
    (2 planes x ci), same 6/8-quadrant kd band as direct (75% PE).
    Winograd replaces 3 kw-taps at N=62 with 4 t-points at N=31
    => 2/3 the PE cycles of the direct kernel.
  - Per pair, 62 output rows = 4 chunks of {16,16,16,14}; per chunk 4
    PSUM banks hold m_0..m_3 [128, rows, 31]; 2 chunks in flight.
  - Drain (inverse transform, fp32): y_even = m0+m1+m2+bias,
    y_odd = m1-m2-m3+bias via DVE, written interleaved (stride 2) into
    SBUF staging, then DMA to DRAM.

Measured baseline (direct bf16): 483 us. This kernel targets ~330 us.
"""

import numpy as np
import ml_dtypes

import concourse.mybir as mybir
import concourse.tile as tile
from concourse import bacc
from concourse.bass_utils import run_bass_kernel_spmd

EPS = 1e-8
N_CORES = 8
N, CIN, COUT, D = 4, 64, 64, 64
DO = D - 2              # 62 output planes/rows/cols
WT = 31                 # Winograd w-tiles per row
T = 4                   # transform points
PLANES_IN = 34          # input planes per core
PAIRS = 16              # output plane pairs per core (32 planes)
SLOTS = PLANES_IN // 2  # 17
XRING = 6               # x~ slot ring buffers
CHUNK_ROWS = (16, 16, 16, 14)
CHUNK_OFF = (0, 16, 32, 48)
BF16 = mybir.dt.bfloat16
F32 = mybir.dt.float32
ADD = mybir.AluOpType.add
SUB = mybir.AluOpType.subtract

_compiled = {}

# stream order per pair: var-major, then kh, then t
_STREAMS = [(var, kh, t) for var in range(2) for kh in range(3)
            for t in range(T)]


def _build_nc():
    nc = bacc.Bacc("TRN2", target_bir_lowering=False, debug=False,
                   num_devices=N_CORES)
    xt_d = nc.dram_tensor("xt", [CIN, PLANES_IN, T, D, WT], BF16,
                          kind="ExternalInput").ap()
    wts = nc.dram_tensor("wts", [128, 24 * 128], BF16,
                         kind="ExternalInput").ap()
    b128 = nc.dram_tensor("b128", [128, 1], F32, kind="ExternalInput").ap()
    y = nc.dram_tensor("y", [COUT, 2 * PAIRS, DO * DO], BF16,
                       kind="ExternalOutput").ap()

    with tile.TileContext(nc) as tc:
        with (
            tc.tile_pool(name="wp", bufs=1) as wpool,
            tc.tile_pool(name="xp", bufs=XRING) as xpool,
            tc.tile_pool(name="ps", bufs=8, space="PSUM") as pspool,
            tc.tile_pool(name="st", bufs=4) as stpool,
            tc.tile_pool(name="tp", bufs=4) as tmppool,
        ):
            w_sb = wpool.tile([128, 24 * 128], BF16)
            # split the weight load across queues — it gates the first matmul
            for i in range(6):
                o = i * 512
                nc.sync.dma_start(w_sb[:, o:o + 512], wts[:, o:o + 512])
            bias_sb = wpool.tile([128, 1], F32)
            nc.sync.dma_start(bias_sb[:, :], b128[:, :])

            slots = {}

            def load_slot(q, groups=None):
                t = xpool.tile([128, T, D, WT], BF16, tag="xt",
                               name=f"xt_{q}")
                slots[q] = t
                if groups is not None:
                    return
                # halves go to the sync/scalar DGE queue sets — one set
                # alone (~190 GB/s) starves the head
                for half, pl in ((0, 2 * q), (1, 2 * q + 1)):
                    p0, p1 = 64 * half, 64 * half + 64
                    eng = nc.sync if half == 0 else nc.scalar
                    for g in range(2):
                        r = 32 * g
                        eng.dma_start(t[p0:p1, :, r:r + 32, :],
                                      xt_d[:, pl, :, r:r + 32, :])

            def load_groups(q, gs):
                t = slots[q]
                for half, pl in ((0, 2 * q), (1, 2 * q + 1)):
                    p0, p1 = 64 * half, 64 * half + 64
                    eng = nc.sync if half == 0 else nc.scalar
                    for g in gs:
                        r = 16 * g
                        eng.dma_start(t[p0:p1, :, r:r + 16, :],
                                      xt_d[:, pl, :, r:r + 16, :])

            # slots 0,1: interleave 16-row groups so pair 0's first chunks
            # have both slots' early rows ASAP
            load_slot(0, groups=())
            load_slot(1, groups=())
            for g in range(4):
                load_groups(0, (g,))
                load_groups(1, (g,))
            for q in range(2, XRING):
                load_slot(q)

            # PE warmup: hold the PE busy with garbage matmuls while the
            # first slots' DMAs are in flight so the HAM clock gate opens
            # (full 2.4 GHz needs ~3.4us of sustained activity).
            warm_src = wpool.tile([128, 496], BF16, name="warm_src")
            nc.vector.memset(warm_src[:, :], 0.0)
            warm_ps = pspool.tile([128, 16, WT], F32, tag="ps",
                                  name="warm_ps")
            for _ in range(250):
                nc.tensor.matmul(warm_ps[:, 0:2, :], warm_src[:, 0:128],
                                 warm_src[:, 128:190], start=True, stop=True)

            IDENT = mybir.ActivationFunctionType.Identity

            def drain(j, c, psums):
                rows = CHUNK_ROWS[c]
                stage = stpool.tile([128, 16, DO], F32, tag="stage")
                stage_bf = stpool.tile([128, 16, DO], BF16, tag="stage_bf")
                t1 = tmppool.tile([128, 16, WT], F32, tag="tmp")
                t2 = tmppool.tile([128, 16, WT], F32, tag="tmp")
                g1 = tmppool.tile([128, 16, WT], F32, tag="tmp")
                g2 = tmppool.tile([128, 16, WT], F32, tag="tmp")
                m0, m1, m2, m3 = psums
                # a = m1 + bias (ACT); b = -m3 (ACT); gpsimd can't read PSUM
                nc.scalar.activation(t1[:, :rows], m1[:, :rows], IDENT,
                                     bias=bias_sb)
                nc.scalar.activation(g1[:, :rows], m3[:, :rows],
                                     mybir.ActivationFunctionType.Copy,
                                     scale=-1.0)
                # y_even = (a + m0) + m2
                nc.vector.tensor_tensor(out=t2[:, :rows], in0=t1[:, :rows],
                                        in1=m0[:, :rows], op=ADD)
                nc.vector.tensor_tensor(out=stage[:, :rows, 0:DO:2],
                                        in0=t2[:, :rows], in1=m2[:, :rows],
                                        op=ADD)
                # y_odd = (a - m2) + b
                nc.vector.tensor_tensor(out=g2[:, :rows], in0=t1[:, :rows],
                                        in1=m2[:, :rows], op=SUB)
                nc.gpsimd.tensor_tensor(out=stage[:, :rows, 1:DO:2],
                                        in0=g2[:, :rows], in1=g1[:, :rows],
                                        op=ADD)
                # contiguous downcast to bf16 (strided bf16 writes fault the
                # exec unit) — halves output DMA bytes
                nc.scalar.activation(stage_bf[:, :rows], stage[:, :rows],
                                     mybir.ActivationFunctionType.Copy)
                # alternate output DMAs between gpsimd's and scalar's DGE
                # queue sets — keeps sync free for inputs and halves the
                # end-of-kernel queue flush
                o = CHUNK_OFF[c] * DO
                n = rows * DO
                eng = nc.gpsimd if c % 2 == 0 else nc.scalar
                eng.dma_start(y[:, 2 * j, o:o + n], stage_bf[0:64, :rows])
                eng.dma_start(y[:, 2 * j + 1, o:o + n],
                              stage_bf[64:128, :rows])

            for j in range(PAIRS):
                last = j == PAIRS - 1
                # chunk-major: drains overlap the next chunk's matmuls
                for c in range(2 if last else 4):
                    psums = [pspool.tile([128, 16, WT], F32, tag="ps",
                                         name=f"ps_{j}_{c}_{t}")
                             for t in range(T)]
                    rows = CHUNK_ROWS[c]
                    for var, kh, t in _STREAMS:
                        blk = (kh * 4 + t) * 2 + var
                        lhsT = w_sb[:, blk * 128:(blk + 1) * 128]
                        sl = slots[j + var]
                        r0 = CHUNK_OFF[c] + kh
                        nc.tensor.matmul(
                            psums[t][:, :rows], lhsT,
                            sl[:, t, r0:r0 + rows, :],
                            start=(var == 0 and kh == 0),
                            stop=(var == 1 and kh == 2))
                    drain(j, c, psums)
                # prefetch the slot this pair's buffer slot frees up
                q = j + XRING
                if q < SLOTS:
                    load_slot(q)
    nc.compile()
    return nc


def _modulated_weights(s_n, style_weight, style_bias, weight):
    st = s_n.astype(np.float32) @ style_weight.T.astype(np.float32) + style_bias
    w = weight * st[None, :, None, None, None]
    demod = 1.0 / np.sqrt(np.sum(w * w, axis=(1, 2, 3, 4)) + EPS)
    return w * demod[:, None, None, None, None]

_G = np.array([[1, 0, 0], [.5, .5, .5], [.5, -.5, .5], [0, 0, 1]], np.float32)


def _build_lhsT(wmod):
    """(24, 128, 128) fp32 blocks: blk=(kh*4+t)*2+var; lhsT[k=(half,ci), m=(colhalf,co)]."""
    ut = np.einsum('tw,oidhw->toidh', _G, wmod)
    out = np.zeros((3, T, 2, 128, 128), np.float32)
    for kh in range(3):
        for t in range(T):
            wt = ut[t][:, :, :, kh]            # (co, ci, kd)
            A = out[kh, t, 0]
            B = out[kh, t, 1]
            A[0:64, 0:64] = wt[:, :, 0].T      # lower -> y[d],   kd0
            A[64:128, 0:64] = wt[:, :, 1].T    # upper -> y[d],   kd1
            A[64:128, 64:128] = wt[:, :, 0].T  # upper -> y[d+1], kd0
            B[0:64, 0:64] = wt[:, :, 2].T      # lower -> y[d],   kd2
            B[0:64, 64:128] = wt[:, :, 1].T    # lower -> y[d+1], kd1
            B[64:128, 64:128] = wt[:, :, 2].T  # upper -> y[d+1], kd2
    return out.reshape(24, 128, 128)


def _wino_xt(xs_bf):
    """(Ci, P, D, D) bf16 -> (Ci, P, 4, D, 31) bf16 W-transform."""
    xw = xs_bf.astype(np.float32)
    i = 2 * np.arange(WT)
    t0 = xw[..., i] - xw[..., i + 2]
    t1 = xw[..., i + 1] + xw[..., i + 2]
    t2 = xw[..., i + 2] - xw[..., i + 1]
    t3 = xw[..., i + 1] - xw[..., i + 3]
    return np.ascontiguousarray(
        np.stack([t0, t1, t2, t3], axis=2).astype(ml_dtypes.bfloat16))


def _prepare_in_maps(x, s, style_weight, style_bias, weight, bias):
    bias128 = np.concatenate([bias.reshape(COUT), bias.reshape(COUT)])
    bias128 = np.ascontiguousarray(bias128.reshape(128, 1), np.float32)

    x_bf = x.astype(ml_dtypes.bfloat16)
    in_maps = []
    for core in range(N_CORES):
        n, half = divmod(core, 2)
        wmod = _modulated_weights(s[n], style_weight, style_bias, weight)
        if half == 0:
            xs = x_bf[n][:, 0:PLANES_IN]
        else:
            # mirrored shard: flip depth + height; kernel taps flip too,
            # so the same program computes the flipped top half
            xs = x_bf[n][:, D - PLANES_IN:D][:, ::-1, ::-1, :]
            wmod = wmod[:, :, ::-1, ::-1, :]
        lhsT = _build_lhsT(np.ascontiguousarray(wmod))  # (24, 128, 128)
        wts = np.ascontiguousarray(
            lhsT.transpose(1, 0, 2).reshape(128, 24 * 128)
        ).astype(ml_dtypes.bfloat16)
        in_maps.append({"xt": _wino_xt(np.ascontiguousarray(xs)),
                        "wts": wts, "b128": bias128})
    return in_maps


def kernel(x, s, style_weight, style_bias, weight, bias):
    x = np.asarray(x)
    s = np.asarray(s)
    style_weight = np.asarray(style_weight, np.float32)
    style_bias = np.asarray(style_bias, np.float32)
    weight = np.asarray(weight, np.float32)
    bias = np.asarray(bias, np.float32)

    if "nc" not in _compiled:
        _compiled["nc"] = _build_nc()
    nc = _compiled["nc"]

    in_maps = _prepare_in_maps(x, s, style_weight, style_bias, weight, bias)
    res = run_bass_kernel_spmd(nc, in_maps, core_ids=list(range(N_CORES)))

    y = np.empty((N, COUT, DO, DO, DO), np.float32)
    for core in range(N_CORES):
        n, half = divmod(core, 2)
        ys = np.asarray(res.results[core]["y"]).astype(np.float32)
        ys = ys.reshape(COUT, 2 * PAIRS, DO, DO)
        if half == 0:
            # planes 0..29 full; planes 30,31 rows 0..31 only
            y[n][:, 0:30] = ys[:, 0:30]
            y[n][:, 30:32, 0:32] = ys[:, 30:32, 0:32]
        else:
            # un-mirror: ysf[p', r'] = global (plane 30+p', row r')
            ysf = ys[:, ::-1, ::-1, :]
            y[n][:, 32:DO] = ysf[:, 2:32]
            y[n][:, 30:32, 32:DO] = ysf[:, 0:2, 32:DO]
    return y
